# revision 2
# baseline (speedup 1.0000x reference)
"""CharCNN embedding kernel for Trainium2 (8 NeuronCores, Bass/Tile).

Computes out[b,t,f] = sum_k conv_w[f, token_ids[b, t+k-pad], k] with zero
padding outside [0,T) — i.e. one_hot(token_ids) -> Conv1d(V->F, k=3, pad=1).

Strategy: data-parallel over batch (B=8 rows, one per core) with the weight
table replicated. Host-side prep (weight relayout + index arithmetic only):
  - per-tap tables table_k [V+1, F] f32 (table_k[v] = conv_w[:, v, k]) with a
    zero row at index V for edge padding.
  - per core b, int16 gather indices ids[k, t] = tok[b, t+k-1] (V when out of
    range), wrapped in the GPSIMD dma_gather layout (idx i at partition i%16,
    slot i//16, replicated across the 8 Q7 cores).
Device per core: for each group of G=8 position-tiles, 3 dma_gather ucode
calls (one per tap, 1024 rows x 2KB each) + 2 DVE adds + HWDGE store into a
[P, NT, F] transposed DRAM layout (contiguous 16KB per partition per store);
host untransposes.
"""

from contextlib import ExitStack

import numpy as np

import concourse.bacc as bacc
import concourse.bass as bass
import concourse.mybir as mybir
import concourse.tile as tile
from concourse._compat import with_exitstack
from concourse.bass_utils import run_bass_kernel_spmd

B = 8
T = 4096
F = 512
V = 32000
VP = V + 1  # +1 zero row per tap
K = 3
P = 128
NT = T // P  # 32 position tiles per core
G = 8  # position tiles per gather instruction
NI = P * G  # num_idxs per gather instruction (1024)
NR = NT // G  # gather rounds (4)
SW = NI // 16  # idx slots per partition per instruction (64)
N_CORES = 8

_nc_cache = {}


@with_exitstack
def _gather_kernel(ctx: ExitStack, tc: tile.TileContext, out_d, tables, idxs_d):
    nc = tc.nc

    idxp = ctx.enter_context(tc.tile_pool(name="idx", bufs=1))
    taps = ctx.enter_context(tc.tile_pool(name="taps", bufs=2))
    outp = ctx.enter_context(tc.tile_pool(name="outp", bufs=2))

    idxs_t = idxp.tile([P, K, NR, SW], mybir.dt.int16)
    nc.gpsimd.dma_start(idxs_t[:], idxs_d[:])

    for r in range(NR):
        tap_tiles = []
        for k in range(K):
            tt = taps.tile([P, G, F], mybir.dt.float32, tag=f"tap{k}")
            nc.gpsimd.dma_gather(
                tt[:],
                tables[k][:],
                idxs_t[:, k, r, :],
                NI,
                NI,
                F,
            )
            tap_tiles.append(tt)
        ot = outp.tile([P, G, F], mybir.dt.float32, tag="out")
        nc.vector.tensor_add(ot[:], tap_tiles[0][:], tap_tiles[1][:])
        nc.vector.tensor_add(ot[:], ot[:], tap_tiles[2][:])
        nc.sync.dma_start(out_d[:, r * G : (r + 1) * G, :], ot[:])


def _build_nc():
    if "nc" in _nc_cache:
        return _nc_cache["nc"]
    nc = bacc.Bacc(
        "TRN2",
        target_bir_lowering=False,
        debug=False,
        enable_asserts=False,
        num_devices=N_CORES,
    )
    tables = [
        nc.dram_tensor(f"table{k}", [VP, F], mybir.dt.float32, kind="ExternalInput").ap()
        for k in range(K)
    ]
    idxs_d = nc.dram_tensor(
        "idxs", [P, K, NR, SW], mybir.dt.int16, kind="ExternalInput"
    ).ap()
    # transposed output layout [P, NT, F]: t = n*P + p
    out_d = nc.dram_tensor(
        "out", [P, NT, F], mybir.dt.float32, kind="ExternalOutput"
    ).ap()
    with tile.TileContext(nc) as tc:
        _gather_kernel(tc, out_d, tables, idxs_d)
    nc.compile()
    _nc_cache["nc"] = nc
    return nc


def _host_prep(token_ids, conv_w):
    wt = np.asarray(conv_w, dtype=np.float32).transpose(2, 1, 0)  # [K, V, F]
    tables = []
    for k in range(K):
        tab = np.empty((VP, F), dtype=np.float32)
        tab[:V] = wt[k]
        tab[V] = 0.0
        tables.append(tab)

    tok = np.asarray(token_ids).astype(np.int64)  # [B, T]
    # ids[b, k, t] = tok[b, t+k-1] or V (zero row) out of range
    ids = np.empty((B, K, T), dtype=np.int16)
    ids[:, 0, 0], ids[:, 0, 1:] = V, tok[:, :-1]
    ids[:, 1, :] = tok
    ids[:, 2, :-1], ids[:, 2, -1] = tok[:, 1:], V
    # dma_gather wrap: per (b, k, r): chunk [NI] -> [SW, 16] -> T, tiled x8
    w = ids.reshape(B, K, NR, SW, 16)
    w = np.ascontiguousarray(w.transpose(0, 1, 2, 4, 3))  # [B, K, NR, 16, SW]
    idxs = np.tile(w[:, None, :, :, :, :], (1, 8, 1, 1, 1, 1))  # [B, 8, K, NR, 16, SW]
    idxs = np.ascontiguousarray(
        idxs.transpose(0, 1, 4, 2, 3, 5).reshape(B, P, K, NR, SW)
    )
    return tables, idxs


def kernel(token_ids, conv_w):
    tables, idxs = _host_prep(token_ids, conv_w)
    nc = _build_nc()
    in_maps = [
        {**{f"table{k}": tables[k] for k in range(K)}, "idxs": idxs[b]}
        for b in range(B)
    ]
    res = run_bass_kernel_spmd(nc, in_maps, core_ids=list(range(N_CORES)))
    # untranspose [P, NT, F] -> [T, F]
    out = np.stack(
        [
            res.results[b]["out"].transpose(1, 0, 2).reshape(T, F)
            for b in range(B)
        ],
        axis=0,
    )
    return np.ascontiguousarray(out, dtype=np.float32)
